# revision 12
# baseline (speedup 1.0000x reference)
"""Multi-head attention (B=4, N=2048, C=1024, H=16, D=64) on 8 TRN2 NeuronCores.

Sharding: core c owns (batch b = c//2, sequence half = c%2) -> 1024 query
tokens, all 16 heads.  Each core computes K and V for its OWN half only;
the partner half arrives via pairwise AllGathers (replica groups [2b, 2b+1]).
Output is purely row-sharded -> host gather is a concat.

Performance notes (v5):
- TRN2's activity-monitor firmware clamps the PE clock to 50% when PE
  activity stays near 100% for more than ~15-30 3.4us windows, and the clamp
  can persist for hundreds of us.  The QKV front is therefore PACED to ~65%
  activity (the attention phase's natural level, measured sustainable):
  each projection group's first matmul is gated on a small Vector-engine
  pace op chained behind the previous group's bias adds.  The front's wall
  time is bounded by the AllGather chain anyway, so pacing costs nothing.
- The four AllGathers are ordered by when their output is consumed
  (K chunk0, V chunk0, V chunk1, K chunk1) and attention iterates m-tiles
  grouped by V chunk, so no dependency is ever on the critical path.
- exp() is split between the Scalar engine (native Exp) and the Vector
  engine (Schraudolph bit-trick: bf16 is the top half of f32, so
  p = bitcast_bf16(int16(S*scale*184.665 + 16250.5)) is one tensor_scalar).
  Scores PSUM + exp are processed in 512-column halves (4 single-bank PSUM
  slots) so the PV matmuls wait on half-tiles, not full tiles.
- Each head's softmax normalization is deferred into the next head's
  iteration stream so the Vector queue never delays an exp.
- All matmuls bf16 with f32 PSUM accumulate.
"""

import numpy as np
import ml_dtypes

import concourse.bass as bass
import concourse.mybir as mybir
import concourse.tile as tile
from concourse import bacc
from concourse.bass import _add_dep_helper
from concourse.bass_utils import run_bass_kernel_spmd

B, N, C = 4, 2048, 1024
H, D = 16, 64
SCALE = D ** -0.5
NCORES = 8
NQ = N // 2          # query tokens per core (own half)
M = N                # key/value tokens after gather

BF16 = mybir.dt.bfloat16
F32 = mybir.dt.float32
I16 = mybir.dt.int16

# Schraudolph exp in bf16: exp(x*SCALE) ~= bitcast_bf16(int16(x*EXPA + EXPB))
EXPA = (2.0 ** 7 / np.log(2.0)) * SCALE
EXPB = 127.0 * 128.0 - 5.5
# which of the 16 m-tile iterations per head run exp on DVE instead of Scalar
DVE_EXP_IDX = {3, 7, 11, 14}
# m-tiles grouped by V gather chunk (j=mt%8: j<4 -> chunk0, j>=4 -> chunk1)
MT_ORDER = [0, 1, 2, 3, 8, 9, 10, 11, 4, 5, 6, 7, 12, 13, 14, 15]
PACE_N = 700         # pace-op length (f32 elems) -> ~0.5us on DVE

_CACHE = {}
LAST_RESULTS = None


def _build():
    nc = bacc.Bacc(
        "TRN2",
        target_bir_lowering=False,
        debug=False,
        enable_asserts=False,
        num_devices=NCORES,
    )
    xoT = nc.dram_tensor("xoT", [C, NQ], BF16, kind="ExternalInput")
    wqkvT = nc.dram_tensor("wqkvT", [C, 3 * C], BF16, kind="ExternalInput")
    bqk = nc.dram_tensor("bqk", [128, 16], F32, kind="ExternalInput")
    bv = nc.dram_tensor("bv", [1, C], BF16, kind="ExternalInput")
    wprojT = nc.dram_tensor("wprojT", [C, C], BF16, kind="ExternalInput")
    bproj = nc.dram_tensor("bproj", [128, 8], F32, kind="ExternalInput")
    yT = nc.dram_tensor("yT", [C, NQ], F32, kind="ExternalOutput")

    groups = [[2 * b, 2 * b + 1] for b in range(B)]

    with tile.TileContext(nc) as tc:
        with (
            tc.tile_pool(name="persist", bufs=1) as pp,
            tc.tile_pool(name="psum", bufs=1, space="PSUM") as psp,
            tc.tile_pool(name="dram", bufs=1, space="DRAM") as dp,
        ):
            lp = tc.alloc_tile_pool(name="front", bufs=1)

            # ---- inputs; wk on the scalar queue so K can start earliest ----
            wk = lp.tile([128, 8, C], BF16, tag="wk", name="wk")
            nc.scalar.dma_start(
                wk[:, :, :],
                wqkvT.rearrange("(c p) o -> p c o", p=128)[:, :, C : 2 * C],
            )
            xo = lp.tile([128, 8, NQ], BF16, tag="xo", name="xo")
            nc.sync.dma_start(xo[:, :, :], xoT.rearrange("(c p) n -> p c n", p=128))
            wv = lp.tile([128, 8, C], BF16, tag="wv", name="wv")
            nc.sync.dma_start(
                wv[:, :, :],
                wqkvT.rearrange("(c p) o -> p c o", p=128)[:, :, 2 * C : 3 * C],
            )
            wq = lp.tile([128, 8, C], BF16, tag="wq", name="wq")
            nc.sync.dma_start(
                wq[:, :, :],
                wqkvT.rearrange("(c p) o -> p c o", p=128)[:, :, 0:C],
            )

            bqk_sb = pp.tile([128, 16], F32, tag="bqk", name="bqk")
            nc.scalar.dma_start(bqk_sb[:, :], bqk[:, :])
            bv_sb = lp.tile([1, C], BF16, tag="bv", name="bv")
            nc.scalar.dma_start(bv_sb[:, :], bv[:, :])
            bp_sb = pp.tile([128, 8], F32, tag="bp", name="bp")
            nc.scalar.dma_start(bp_sb[:, :], bproj[:, :])

            bvb = lp.tile([128, C], BF16, tag="bvb", name="bvb")
            nc.gpsimd.partition_broadcast(bvb[:, :], bv_sb[:, :])

            # ---- persistent attention operands ----
            KT = pp.tile([128, 8, M], BF16, tag="KT", name="KT")
            QT = pp.tile([128, 8, NQ], BF16, tag="QT", name="QT")
            Vb = [
                pp.tile([128, 2, 4, H, D + 1], BF16, tag=f"Vb{c}", name=f"Vb{c}")
                for c in range(2)
            ]
            A_sb = [
                pp.tile([128, NQ], BF16, tag=f"a{i}", name=f"a{i}") for i in range(8)
            ]

            # staging SBUF + DRAM bounce buffers
            kh = lp.tile([128, 8, NQ], BF16, tag="kh", name="kh")
            vh = lp.tile([128, 8, H, D + 1], BF16, tag="vh", name="vh")
            k_in = [dp.tile([512, NQ], BF16, tag=f"ki{c}", name=f"ki{c}") for c in range(2)]
            k_out = [
                dp.tile([2, 512, NQ], BF16, tag=f"ko{c}", name=f"ko{c}") for c in range(2)
            ]
            v_in = [
                dp.tile([512, H * (D + 1)], BF16, tag=f"vi{c}", name=f"vi{c}")
                for c in range(2)
            ]
            v_out = [
                dp.tile([2, 512, H * (D + 1)], BF16, tag=f"vo{c}", name=f"vo{c}")
                for c in range(2)
            ]

            # ---- HAM pacing machinery ----
            pace_sb = lp.tile([1, 2 * PACE_N], F32, tag="pace", name="pace")
            nc.vector.memset(pace_sb[:, :], 0.0)
            pace_state = {"last": None, "flip": 0}

            def pace_group(first_mm, last_dve):
                # gate this group's first matmul on the previous group's pace
                # op; chain a new pace op behind this group's bias adds.
                if pace_state["last"] is not None:
                    _add_dep_helper(
                        first_mm.ins, pace_state["last"].ins, sync=True,
                        reason="HAM activity pacing",
                    )
                f = pace_state["flip"]
                pace_state["flip"] = 1 - f
                pace_state["last"] = nc.vector.tensor_copy(
                    pace_sb[:, f * PACE_N : (f + 1) * PACE_N],
                    pace_sb[:, (1 - f) * PACE_N : (2 - f) * PACE_N],
                )
                # anchor the pace op behind this group's bias adds so the
                # scheduler cannot hoist the pace chain to the start
                _add_dep_helper(
                    pace_state["last"].ins, last_dve.ins, sync=False,
                    reason="HAM pacing anchor",
                )

            def k_heads(c, which):
                # K/Q output channels i*128..(i+1)*128 for own tokens; bias
                # fused into the PSUM->SBUF copy (split in halves for pacing).
                w_sb, boff, dst = (
                    (wk, 8, kh) if which == "k" else (wq, 0, None)
                )
                for i in range(4 * c, 4 * c + 4):
                    ps = psp.tile([128, 2, 512], F32, tag="mm", bufs=2, name="psk")
                    first = None
                    for ct in range(8):
                        for nch in range(2):
                            mm = nc.tensor.matmul(
                                ps[:, nch, :],
                                w_sb[:, ct, i * 128 : (i + 1) * 128],
                                xo[:, ct, nch * 512 : (nch + 1) * 512],
                                start=(ct == 0),
                                stop=(ct == 7),
                            )
                            if first is None:
                                first = mm
                    adds = []
                    for nch in range(2):
                        out_ap = (
                            kh[:, i, nch * 512 : (nch + 1) * 512]
                            if which == "k"
                            else QT[:, i, nch * 512 : (nch + 1) * 512]
                        )
                        adds.append(
                            nc.vector.tensor_scalar_add(
                                out_ap, ps[:, nch, :], bqk_sb[:, boff + i : boff + i + 1]
                            )
                        )
                    pace_group(first, adds[-1])
                if which == "k":
                    nc.sync.dma_start(
                        k_in[c].rearrange("(i p) m -> p i m", p=128),
                        kh[:, 4 * c : 4 * c + 4, :],
                    )
                    nc.gpsimd.collective_compute(
                        "AllGather",
                        mybir.AluOpType.bypass,
                        replica_groups=groups,
                        ins=[k_in[c].opt()],
                        outs=[k_out[c].opt()],
                    )

            def v_tiles(c):
                # V for own token tiles j (all 16 heads); ones column at d=D
                # drives the softmax denominator in PV.
                for j in range(4 * c, 4 * c + 4):
                    ps = psp.tile([128, 2, 8, D], F32, tag="mm", bufs=2, name="psv")
                    first = None
                    for ct in range(8):
                        for vch in range(2):
                            mm = nc.tensor.matmul(
                                ps[:, vch, :, :],
                                xo[:, ct, j * 128 : (j + 1) * 128],
                                wv[:, ct, vch * 512 : (vch + 1) * 512],
                                start=(ct == 0),
                                stop=(ct == 7),
                            )
                            if first is None:
                                first = mm
                    nc.vector.memset(vh[:, j, :, D : D + 1], 1.0)
                    adds = []
                    for vch in range(2):
                        adds.append(
                            nc.vector.tensor_tensor(
                                vh[:, j, vch * 8 : (vch + 1) * 8, 0:D],
                                ps[:, vch, :, :],
                                bvb[:, vch * 512 : (vch + 1) * 512].rearrange(
                                    "p (h e) -> p h e", e=D
                                ),
                                op=mybir.AluOpType.add,
                            )
                        )
                    pace_group(first, adds[-1])
                nc.sync.dma_start(
                    v_in[c].rearrange("(j p) f -> p j f", p=128),
                    vh[:, 4 * c : 4 * c + 4, :, :].rearrange("p j h e -> p j (h e)"),
                )
                nc.gpsimd.collective_compute(
                    "AllGather",
                    mybir.AluOpType.bypass,
                    replica_groups=groups,
                    ins=[v_in[c].opt()],
                    outs=[v_out[c].opt()],
                )

            # gather order = consumption order: K chunk0 at attention start,
            # V chunk0 ~8 iters in, V chunk1 ~8 iters later, K chunk1 only
            # from head 8 (~140us later).
            k_heads(0, "k")
            v_tiles(0)
            v_tiles(1)
            k_heads(1, "k")

            # ---- unstage gathered K chunk0 (scalar queue) ----
            for r in range(2):
                nc.scalar.dma_start(
                    KT[:, 0:4, r * NQ : (r + 1) * NQ],
                    k_out[0][r].rearrange("(i p) m -> p i m", p=128),
                )
            # V unstages ride the gpsimd queue (SWDGE)
            for c in range(2):
                for r in range(2):
                    nc.gpsimd.dma_start(
                        Vb[c][:, r, :, :, :].rearrange("p j h e -> p j (h e)"),
                        v_out[c][r].rearrange("(j p) f -> p j f", p=128),
                    )
            # K chunk1 unstage + proj weights on the sync queue (idle then)
            for r in range(2):
                nc.sync.dma_start(
                    KT[:, 4:8, r * NQ : (r + 1) * NQ],
                    k_out[1][r].rearrange("(i p) m -> p i m", p=128),
                )

            # ---- Q (own tokens; paced like K) ----
            k_heads(0, "q")
            k_heads(1, "q")

            lp.release()
            wk2 = tc.alloc_tile_pool(name="attnwork", bufs=1)
            wp_sb = wk2.tile([128, 8, C], BF16, tag="wp", name="wp")
            nc.sync.dma_start(
                wp_sb[:, :, :], wprojT.rearrange("(c p) o -> p c o", p=128)
            )

            # ---- attention ----
            # Head-PAIR processing: heads (2i, 2i+1) live on partitions 0-63 /
            # 64-127 of KT/QT.  Their score matmuls are 64-contraction, so the
            # PE row-tiles them ((0,0) / (64,0) auto-derived) and runs them
            # CONCURRENTLY - one 512-cycle pass yields S^T tiles for BOTH
            # heads.  The n range is split in halves (nch) processed
            # sequentially per pair so PSUM fits: 4 score banks + 2 live PV
            # accumulators + 2 pending-norm accumulators = 8.
            # PV accumulates over all 16 m-tiles into [65,512] (ones column
            # of V gives the softmax denominator in row 64).  Normalization
            # of a pair-half is deferred into the next half's stream.
            pending = []

            def norm_a(ent):
                # stage PV out of PSUM, extract denominator, reciprocal
                pva, pvb = ent["pva"], ent["pvb"]
                stage = wk2.tile([65, 2, 512], BF16, tag="st", bufs=3, name="stage")
                den = wk2.tile([1, 2, 512], F32, tag="den", bufs=2, name="den")
                for j, pv in ((0, pva), (1, pvb)):
                    nc.vector.tensor_copy(stage[:, j, :], pv[:, :])
                    nc.vector.tensor_copy(den[:, j, :], pv[64:65, :])
                rcp = wk2.tile([1, 2, 512], F32, tag="rcp", bufs=2, name="rcp")
                nc.vector.reciprocal_approx_fast(rcp[:, :, :], den[:, :, :])
                rb = wk2.tile([64, 2, 512], F32, tag="rb", bufs=2, name="rb")
                nc.gpsimd.partition_broadcast(
                    rb.rearrange("p j n -> p (j n)"),
                    rcp.rearrange("p j n -> p (j n)"),
                )
                ent["stage"], ent["rb"] = stage, rb

            def norm_b(ent):
                pr, nch = ent["pr"], ent["nch"]
                ncs = slice(nch * 512, (nch + 1) * 512)
                for j in range(2):
                    nc.vector.tensor_mul(
                        A_sb[pr][j * 64 : j * 64 + 64, ncs],
                        ent["stage"][0:64, j, :],
                        ent["rb"][:, j, :],
                    )

            for pr in range(H // 2):
                for nch in range(2):
                    ncs = slice(nch * 512, (nch + 1) * 512)
                    pva = psp.tile([65, 512], F32, tag="acca", bufs=2, name="pva")
                    pvb = psp.tile([65, 512], F32, tag="accb", bufs=2, name="pvb")

                    def emit_pv(ent):
                        # PV runs TWO iterations behind scores/exp so its exp
                        # semaphores are satisfied well before decode time
                        # (the PE queue then prefetches and avoids pipeline
                        # flushes).
                        mt, idx, p = ent
                        r, j = mt // 8, mt % 8
                        vc, vj = j // 4, j % 4
                        for pv, h in ((pva, 2 * pr), (pvb, 2 * pr + 1)):
                            nc.tensor.matmul(
                                pv[:, :],
                                Vb[vc][:, r, vj, h, :],
                                p[:, h % 2, :],
                                start=(idx == 0),
                                stop=(idx == 15),
                                skip_group_check=True,
                            )

                    # Per iteration: the score PAIR writes a single 2-bank
                    # PSUM tile, so ONE exp instruction covers both heads
                    # (halving engine fixed costs and semaphore traffic -
                    # every sem-carrying instruction costs ~SEM_DELAY=100ns
                    # of PE pipeline refill).  PV runs two iterations behind
                    # so its exp semaphores are satisfied at decode time.
                    inflight = []
                    for idx, mt in enumerate(MT_ORDER):
                        sp = psp.tile([128, 2, 512], F32, tag="mm", bufs=2, name="pss")
                        p = wk2.tile([128, 2, 512], BF16, tag="p", bufs=6, name="p")
                        for j in range(2):
                            nc.tensor.matmul(
                                sp[:, j, :],
                                KT[j * 64 : j * 64 + 64, pr, mt * 128 : (mt + 1) * 128],
                                QT[j * 64 : j * 64 + 64, pr, ncs],
                                start=True,
                                stop=True,
                            )
                        # ~10/16 exp batches on ACT, 6/16 on DVE balances the
                        # engines against the shortened PE pipeline
                        if idx in (2, 5, 7, 10, 13, 15):
                            nc.vector.tensor_scalar(
                                p.bitcast(I16),
                                sp[:, :, :],
                                EXPA, EXPB,
                                op0=mybir.AluOpType.mult,
                                op1=mybir.AluOpType.add,
                            )
                        else:
                            nc.scalar.activation(
                                p[:, :, :], sp[:, :, :],
                                mybir.ActivationFunctionType.Exp, scale=SCALE,
                            )
                        if len(inflight) == 2:
                            emit_pv(inflight.pop(0))
                        inflight.append((mt, idx, p))
                        if idx == 4 and pending:
                            norm_a(pending[0])
                        if idx == 9 and pending:
                            norm_b(pending.pop(0))
                    for ent in inflight:
                        emit_pv(ent)
                    pending.append({"pr": pr, "nch": nch, "pva": pva, "pvb": pvb})
            while pending:
                ent = pending.pop(0)
                norm_a(ent)
                norm_b(ent)

            # ---- output projection (ot pairs: 4 open accumulators) ----
            for op2 in range(4):
                pss = [
                    psp.tile([128, 512], F32, tag=("acca", "accb")[nch], bufs=2, name="psp")
                    for j in range(2)
                    for nch in range(2)
                ]
                for dd in range(8):
                    for j in range(2):
                        ot = op2 * 2 + j
                        for nch in range(2):
                            nc.tensor.matmul(
                                pss[j * 2 + nch][:, :],
                                wp_sb[:, dd, ot * 128 : (ot + 1) * 128],
                                A_sb[dd][:, nch * 512 : (nch + 1) * 512],
                                start=(dd == 0),
                                stop=(dd == 7),
                            )
                for j in range(2):
                    ot = op2 * 2 + j
                    for nch in range(2):
                        y = wk2.tile([128, 512], F32, tag="y", bufs=3, name="y")
                        nc.vector.tensor_scalar_add(
                            y[:, :], pss[j * 2 + nch][:, :],
                            bp_sb[:, ot : ot + 1],
                        )
                        nc.scalar.dma_start(
                            yT[ot * 128 : (ot + 1) * 128, nch * 512 : (nch + 1) * 512],
                            y[:, :],
                        )
            wk2.release()

    nc.compile()
    return nc


def kernel(x, w_qkv, b_qkv, w_proj, b_proj):
    global LAST_RESULTS
    bf = ml_dtypes.bfloat16
    x = np.asarray(x, np.float32)
    w_qkv = np.asarray(w_qkv, np.float32)
    b_qkv = np.asarray(b_qkv, np.float32)
    w_proj = np.asarray(w_proj, np.float32)
    b_proj = np.asarray(b_proj, np.float32)

    wqkvT = np.ascontiguousarray(w_qkv.T.astype(bf))            # [1024, 3072]
    wprojT = np.ascontiguousarray(w_proj.T.astype(bf))          # [1024, 1024]
    bqk = np.ascontiguousarray(
        b_qkv[: 2 * C].reshape(16, 128).T.astype(np.float32)
    )                                                           # [128, 16]
    bv = np.ascontiguousarray(b_qkv[None, 2 * C :].astype(bf))  # [1, 1024]
    bproj = np.ascontiguousarray(
        b_proj.reshape(8, 128).T.astype(np.float32)
    )                                                           # [128, 8]

    in_maps = []
    for core in range(NCORES):
        b, half = core // 2, core % 2
        own = x[b][half * NQ : (half + 1) * NQ]                 # [1024, 1024]
        in_maps.append(
            {
                "xoT": np.ascontiguousarray(own.T.astype(bf)),
                "wqkvT": wqkvT,
                "bqk": bqk,
                "bv": bv,
                "wprojT": wprojT,
                "bproj": bproj,
            }
        )

    if "nc" not in _CACHE:
        _CACHE["nc"] = _build()
    nc = _CACHE["nc"]

    res = run_bass_kernel_spmd(nc, in_maps, core_ids=list(range(NCORES)))
    LAST_RESULTS = res

    out = np.empty((B, N, C), np.float32)
    for core in range(NCORES):
        b, half = core // 2, core % 2
        out[b, half * NQ : (half + 1) * NQ, :] = res.results[core]["yT"].T
    return out


if __name__ == "__main__":
    rng = np.random.default_rng(0)
    s = C ** -0.5
    ins = {
        "x": rng.standard_normal((B, N, C)).astype(np.float32),
        "w_qkv": (rng.standard_normal((3 * C, C)) * s).astype(np.float32),
        "b_qkv": (rng.standard_normal(3 * C) * 0.02).astype(np.float32),
        "w_proj": (rng.standard_normal((C, C)) * s).astype(np.float32),
        "b_proj": (rng.standard_normal(C) * 0.02).astype(np.float32),
    }
    y = kernel(**ins)
    print("out", y.shape, y.dtype, float(np.abs(y).mean()))



# revision 17
# speedup vs baseline: 1.1012x; 1.1012x over previous
"""Multi-head attention (B=4, N=2048, C=1024, H=16, D=64) on 8 TRN2 NeuronCores.

Sharding: core c owns (batch b = c//2, sequence half = c%2) -> 1024 query
tokens, all 16 heads.  Each core computes K and V for its OWN half only;
the partner half arrives via pairwise AllGathers (replica groups [2b, 2b+1]).
Output is purely row-sharded -> host gather is a concat.

Performance notes (v5):
- TRN2's activity-monitor firmware clamps the PE clock to 50% when PE
  activity stays near 100% for more than ~15-30 3.4us windows, and the clamp
  can persist for hundreds of us.  The QKV front is therefore PACED to ~65%
  activity (the attention phase's natural level, measured sustainable):
  each projection group's first matmul is gated on a small Vector-engine
  pace op chained behind the previous group's bias adds.  The front's wall
  time is bounded by the AllGather chain anyway, so pacing costs nothing.
- The four AllGathers are ordered by when their output is consumed
  (K chunk0, V chunk0, V chunk1, K chunk1) and attention iterates m-tiles
  grouped by V chunk, so no dependency is ever on the critical path.
- exp() is split between the Scalar engine (native Exp) and the Vector
  engine (Schraudolph bit-trick: bf16 is the top half of f32, so
  p = bitcast_bf16(int16(S*scale*184.665 + 16250.5)) is one tensor_scalar).
  Scores PSUM + exp are processed in 512-column halves (4 single-bank PSUM
  slots) so the PV matmuls wait on half-tiles, not full tiles.
- Each head's softmax normalization is deferred into the next head's
  iteration stream so the Vector queue never delays an exp.
- All matmuls bf16 with f32 PSUM accumulate.
"""

import numpy as np
import ml_dtypes

import concourse.bass as bass
import concourse.mybir as mybir
import concourse.tile as tile
from concourse import bacc
from concourse.bass import _add_dep_helper
from concourse.bass_utils import run_bass_kernel_spmd

B, N, C = 4, 2048, 1024
H, D = 16, 64
SCALE = D ** -0.5
NCORES = 8
NQ = N // 2          # query tokens per core (own half)
M = N                # key/value tokens after gather

BF16 = mybir.dt.bfloat16
F32 = mybir.dt.float32
I16 = mybir.dt.int16

# Schraudolph exp in bf16: exp(x*SCALE) ~= bitcast_bf16(int16(x*EXPA + EXPB))
EXPA = (2.0 ** 7 / np.log(2.0)) * SCALE
EXPB = 127.0 * 128.0 - 5.5
# which of the 16 m-tile iterations per head run exp on DVE instead of Scalar
DVE_EXP_IDX = {3, 7, 11, 14}
# m-tiles grouped by V gather chunk (j=mt%8: j<4 -> chunk0, j>=4 -> chunk1)
MT_ORDER = [0, 1, 2, 3, 8, 9, 10, 11, 4, 5, 6, 7, 12, 13, 14, 15]
PACE_N = 700         # pace-op length (f32 elems) -> ~0.5us on DVE

_CACHE = {}
LAST_RESULTS = None


def _build():
    nc = bacc.Bacc(
        "TRN2",
        target_bir_lowering=False,
        debug=False,
        enable_asserts=False,
        num_devices=NCORES,
    )
    xoT = nc.dram_tensor("xoT", [C, NQ], BF16, kind="ExternalInput")
    wqkvT = nc.dram_tensor("wqkvT", [C, 3 * C], BF16, kind="ExternalInput")
    bqk = nc.dram_tensor("bqk", [128, 16], F32, kind="ExternalInput")
    bv = nc.dram_tensor("bv", [1, C], BF16, kind="ExternalInput")
    wprojT = nc.dram_tensor("wprojT", [C, C], BF16, kind="ExternalInput")
    bproj = nc.dram_tensor("bproj", [128, 8], F32, kind="ExternalInput")
    yT = nc.dram_tensor("yT", [C, NQ], F32, kind="ExternalOutput")

    groups = [[2 * b, 2 * b + 1] for b in range(B)]

    with tile.TileContext(nc) as tc:
        with (
            tc.tile_pool(name="persist", bufs=1) as pp,
            tc.tile_pool(name="psum", bufs=1, space="PSUM") as psp,
            tc.tile_pool(name="dram", bufs=1, space="DRAM") as dp,
        ):
            lp = tc.alloc_tile_pool(name="front", bufs=1)

            # ---- inputs; wk on the scalar queue so K can start earliest ----
            wk = lp.tile([128, 8, C], BF16, tag="wk", name="wk")
            nc.scalar.dma_start(
                wk[:, :, :],
                wqkvT.rearrange("(c p) o -> p c o", p=128)[:, :, C : 2 * C],
            )
            xo = lp.tile([128, 8, NQ], BF16, tag="xo", name="xo")
            nc.sync.dma_start(xo[:, :, :], xoT.rearrange("(c p) n -> p c n", p=128))
            wv = lp.tile([128, 8, C], BF16, tag="wv", name="wv")
            nc.sync.dma_start(
                wv[:, :, :],
                wqkvT.rearrange("(c p) o -> p c o", p=128)[:, :, 2 * C : 3 * C],
            )
            wq = lp.tile([128, 8, C], BF16, tag="wq", name="wq")
            nc.sync.dma_start(
                wq[:, :, :],
                wqkvT.rearrange("(c p) o -> p c o", p=128)[:, :, 0:C],
            )

            bqk_sb = pp.tile([128, 16], F32, tag="bqk", name="bqk")
            nc.scalar.dma_start(bqk_sb[:, :], bqk[:, :])
            bv_sb = lp.tile([1, C], BF16, tag="bv", name="bv")
            nc.scalar.dma_start(bv_sb[:, :], bv[:, :])
            bp_sb = pp.tile([128, 8], F32, tag="bp", name="bp")
            nc.scalar.dma_start(bp_sb[:, :], bproj[:, :])

            bvb = lp.tile([128, C], BF16, tag="bvb", name="bvb")
            nc.gpsimd.partition_broadcast(bvb[:, :], bv_sb[:, :])

            # ---- persistent attention operands ----
            KT = pp.tile([128, 8, M], BF16, tag="KT", name="KT")
            QT = pp.tile([128, 8, NQ], BF16, tag="QT", name="QT")
            Vb = [
                pp.tile([128, 2, 4, H, D + 1], BF16, tag=f"Vb{c}", name=f"Vb{c}")
                for c in range(2)
            ]
            A_sb = [
                pp.tile([128, NQ], BF16, tag=f"a{i}", name=f"a{i}") for i in range(8)
            ]

            # staging SBUF + DRAM bounce buffers
            kh = lp.tile([128, 8, NQ], BF16, tag="kh", name="kh")
            vh = lp.tile([128, 8, H, D + 1], BF16, tag="vh", name="vh")
            k_in = [dp.tile([512, NQ], BF16, tag=f"ki{c}", name=f"ki{c}") for c in range(2)]
            k_out = [
                dp.tile([2, 512, NQ], BF16, tag=f"ko{c}", name=f"ko{c}") for c in range(2)
            ]
            v_in = [
                dp.tile([512, H * (D + 1)], BF16, tag=f"vi{c}", name=f"vi{c}")
                for c in range(2)
            ]
            v_out = [
                dp.tile([2, 512, H * (D + 1)], BF16, tag=f"vo{c}", name=f"vo{c}")
                for c in range(2)
            ]

            # ---- HAM pacing machinery ----
            pace_sb = lp.tile([1, 2 * PACE_N], F32, tag="pace", name="pace")
            nc.vector.memset(pace_sb[:, :], 0.0)
            pace_state = {"last": None, "flip": 0}

            def pace_group(first_mm, last_dve):
                # gate this group's first matmul on the previous group's pace
                # op; chain a new pace op behind this group's bias adds.
                if pace_state["last"] is not None:
                    _add_dep_helper(
                        first_mm.ins, pace_state["last"].ins, sync=True,
                        reason="HAM activity pacing",
                    )
                f = pace_state["flip"]
                pace_state["flip"] = 1 - f
                pace_state["last"] = nc.vector.tensor_copy(
                    pace_sb[:, f * PACE_N : (f + 1) * PACE_N],
                    pace_sb[:, (1 - f) * PACE_N : (2 - f) * PACE_N],
                )
                # anchor the pace op behind this group's bias adds so the
                # scheduler cannot hoist the pace chain to the start
                _add_dep_helper(
                    pace_state["last"].ins, last_dve.ins, sync=False,
                    reason="HAM pacing anchor",
                )

            def k_heads(c, which):
                # K/Q output channels i*128..(i+1)*128 for own tokens; bias
                # fused into the PSUM->SBUF copy (split in halves for pacing).
                w_sb, boff, dst = (
                    (wk, 8, kh) if which == "k" else (wq, 0, None)
                )
                for i in range(4 * c, 4 * c + 4):
                    ps = [
                        psp.tile([128, 512], F32, tag="mm", bufs=4, name="psk")
                        for _ in range(2)
                    ]
                    first = None
                    for ct in range(8):
                        for nch in range(2):
                            mm = nc.tensor.matmul(
                                ps[nch][:, :],
                                w_sb[:, ct, i * 128 : (i + 1) * 128],
                                xo[:, ct, nch * 512 : (nch + 1) * 512],
                                start=(ct == 0),
                                stop=(ct == 7),
                            )
                            if first is None:
                                first = mm
                    adds = []
                    for nch in range(2):
                        out_ap = (
                            kh[:, i, nch * 512 : (nch + 1) * 512]
                            if which == "k"
                            else QT[:, i, nch * 512 : (nch + 1) * 512]
                        )
                        adds.append(
                            nc.vector.tensor_scalar_add(
                                out_ap, ps[nch][:, :], bqk_sb[:, boff + i : boff + i + 1]
                            )
                        )
                    pace_group(first, adds[-1])
                if which == "k":
                    nc.sync.dma_start(
                        k_in[c].rearrange("(i p) m -> p i m", p=128),
                        kh[:, 4 * c : 4 * c + 4, :],
                    )
                    nc.gpsimd.collective_compute(
                        "AllGather",
                        mybir.AluOpType.bypass,
                        replica_groups=groups,
                        ins=[k_in[c].opt()],
                        outs=[k_out[c].opt()],
                    )

            def v_tiles(c):
                # V for own token tiles j (all 16 heads); ones column at d=D
                # drives the softmax denominator in PV.
                for j in range(4 * c, 4 * c + 4):
                    ps = [
                        psp.tile([128, 8, D], F32, tag="mm", bufs=4, name="psv")
                        for _ in range(2)
                    ]
                    first = None
                    for ct in range(8):
                        for vch in range(2):
                            mm = nc.tensor.matmul(
                                ps[vch][:, :, :],
                                xo[:, ct, j * 128 : (j + 1) * 128],
                                wv[:, ct, vch * 512 : (vch + 1) * 512],
                                start=(ct == 0),
                                stop=(ct == 7),
                            )
                            if first is None:
                                first = mm
                    nc.vector.memset(vh[:, j, :, D : D + 1], 1.0)
                    adds = []
                    for vch in range(2):
                        adds.append(
                            nc.vector.tensor_tensor(
                                vh[:, j, vch * 8 : (vch + 1) * 8, 0:D],
                                ps[vch][:, :, :],
                                bvb[:, vch * 512 : (vch + 1) * 512].rearrange(
                                    "p (h e) -> p h e", e=D
                                ),
                                op=mybir.AluOpType.add,
                            )
                        )
                    pace_group(first, adds[-1])
                nc.sync.dma_start(
                    v_in[c].rearrange("(j p) f -> p j f", p=128),
                    vh[:, 4 * c : 4 * c + 4, :, :].rearrange("p j h e -> p j (h e)"),
                )
                nc.gpsimd.collective_compute(
                    "AllGather",
                    mybir.AluOpType.bypass,
                    replica_groups=groups,
                    ins=[v_in[c].opt()],
                    outs=[v_out[c].opt()],
                )

            # gather order = consumption order: K chunk0 at attention start,
            # V chunk0 ~8 iters in, V chunk1 ~8 iters later, K chunk1 only
            # from head 8 (~140us later).
            k_heads(0, "k")
            v_tiles(0)
            v_tiles(1)
            k_heads(1, "k")

            # ---- unstage gathered K chunk0 (scalar queue) ----
            for r in range(2):
                nc.scalar.dma_start(
                    KT[:, 0:4, r * NQ : (r + 1) * NQ],
                    k_out[0][r].rearrange("(i p) m -> p i m", p=128),
                )
            # V unstages ride the gpsimd queue (SWDGE)
            for c in range(2):
                for r in range(2):
                    nc.gpsimd.dma_start(
                        Vb[c][:, r, :, :, :].rearrange("p j h e -> p j (h e)"),
                        v_out[c][r].rearrange("(j p) f -> p j f", p=128),
                    )
            # K chunk1 unstage + proj weights on the sync queue (idle then)
            for r in range(2):
                nc.sync.dma_start(
                    KT[:, 4:8, r * NQ : (r + 1) * NQ],
                    k_out[1][r].rearrange("(i p) m -> p i m", p=128),
                )

            # ---- Q (own tokens; paced like K) ----
            k_heads(0, "q")
            k_heads(1, "q")

            lp.release()
            wk2 = tc.alloc_tile_pool(name="attnwork", bufs=1)
            wp_sb = wk2.tile([128, 8, C], BF16, tag="wp", name="wp")
            nc.sync.dma_start(
                wp_sb[:, :, :], wprojT.rearrange("(c p) o -> p c o", p=128)
            )

            # ---- attention ----
            # Head-PAIR processing: heads (2i, 2i+1) live on partitions 0-63 /
            # 64-127 of KT/QT.  Their score matmuls are 64-contraction, so the
            # PE row-tiles them ((0,0) / (64,0) auto-derived) and runs them
            # CONCURRENTLY - one 512-cycle pass yields S^T tiles for BOTH
            # heads.  The n range is split in halves (nch) processed
            # sequentially per pair so PSUM fits: 4 score banks + 2 live PV
            # accumulators + 2 pending-norm accumulators = 8.
            # PV accumulates over all 16 m-tiles into [65,512] (ones column
            # of V gives the softmax denominator in row 64).  Normalization
            # of a pair-half is deferred into the next half's stream.
            # Normalization of a finished pair-half is split into 8 small
            # steps run one-per-iteration inside the NEXT half's stream, so
            # the DVE queue never builds up enough backlog to delay an exp
            # past the decode time of the matmul waiting on it (a late
            # semaphore costs the PE a ~110ns pipeline refill per matmul).
            norm_thunks = []

            def queue_norm(ent):
                def t_stage_a():
                    ent["stage"] = wk2.tile(
                        [65, 2, 512], BF16, tag="st", bufs=3, name="stage"
                    )
                    nc.vector.tensor_copy(ent["stage"][:, 0, :], ent["pva"][:, :])

                def t_stage_b():
                    nc.vector.tensor_copy(ent["stage"][:, 1, :], ent["pvb"][:, :])

                def t_den():
                    ent["den"] = wk2.tile(
                        [1, 2, 512], F32, tag="den", bufs=2, name="den"
                    )
                    nc.vector.tensor_copy(ent["den"][:, 0, :], ent["pva"][64:65, :])
                    nc.vector.tensor_copy(ent["den"][:, 1, :], ent["pvb"][64:65, :])

                def t_rcp_a():
                    ent["rcp"] = wk2.tile(
                        [1, 2, 512], F32, tag="rcp", bufs=2, name="rcp"
                    )
                    nc.vector.reciprocal_approx_fast(
                        ent["rcp"][:, 0, :], ent["den"][:, 0, :]
                    )

                def t_rcp_b():
                    nc.vector.reciprocal_approx_fast(
                        ent["rcp"][:, 1, :], ent["den"][:, 1, :]
                    )

                def t_bcast():
                    ent["rb"] = wk2.tile(
                        [64, 2, 512], F32, tag="rb", bufs=2, name="rb"
                    )
                    nc.gpsimd.partition_broadcast(
                        ent["rb"].rearrange("p j n -> p (j n)"),
                        ent["rcp"].rearrange("p j n -> p (j n)"),
                    )

                def t_mul(j):
                    def f():
                        ncs2 = slice(ent["nch"] * 512, (ent["nch"] + 1) * 512)
                        nc.vector.tensor_mul(
                            A_sb[ent["pr"]][j * 64 : j * 64 + 64, ncs2],
                            ent["stage"][0:64, j, :],
                            ent["rb"][:, j, :],
                        )
                    return f

                norm_thunks.extend(
                    [t_stage_a, t_stage_b, t_den, t_rcp_a, t_rcp_b,
                     t_bcast, t_mul(0), t_mul(1)]
                )

            for pr in range(H // 2):
                for nch in range(2):
                    ncs = slice(nch * 512, (nch + 1) * 512)
                    pva = psp.tile([65, 512], F32, tag="acca", bufs=2, name="pva")
                    pvb = psp.tile([65, 512], F32, tag="accb", bufs=2, name="pvb")

                    def emit_pv(ent):
                        # PV runs TWO iterations behind scores/exp so its exp
                        # semaphores are satisfied well before decode time
                        # (the PE queue then prefetches and avoids pipeline
                        # flushes).
                        mt, idx, p = ent
                        r, j = mt // 8, mt % 8
                        vc, vj = j // 4, j % 4
                        for pv, h in ((pva, 2 * pr), (pvb, 2 * pr + 1)):
                            nc.tensor.matmul(
                                pv[:, :],
                                Vb[vc][:, r, vj, h, :],
                                p[:, h % 2, :],
                                start=(idx == 0),
                                stop=(idx == 15),
                                skip_group_check=True,
                            )

                    inflight = []
                    for idx, mt in enumerate(MT_ORDER):
                        sp = [
                            psp.tile([128, 512], F32, tag="mm", bufs=4, name="pss")
                            for _ in range(2)
                        ]
                        p = wk2.tile([128, 2, 512], BF16, tag="p", bufs=6, name="p")
                        for j in range(2):
                            nc.tensor.matmul(
                                sp[j][:, :],
                                KT[j * 64 : j * 64 + 64, pr, mt * 128 : (mt + 1) * 128],
                                QT[j * 64 : j * 64 + 64, pr, ncs],
                                start=True,
                                stop=True,
                            )
                        for j in range(2):
                            # ~19/32 tiles on ACT, 13/32 on DVE balances the
                            # engines against the PE pipeline
                            if j == 1 and idx not in (0, 5, 10):
                                nc.vector.tensor_scalar(
                                    p[:, j, :].bitcast(I16),
                                    sp[j][:, :],
                                    EXPA, EXPB,
                                    op0=mybir.AluOpType.mult,
                                    op1=mybir.AluOpType.add,
                                )
                            else:
                                nc.scalar.activation(
                                    p[:, j, :], sp[j][:, :],
                                    mybir.ActivationFunctionType.Exp, scale=SCALE,
                                )
                        if len(inflight) == 2:
                            emit_pv(inflight.pop(0))
                        inflight.append((mt, idx, p))
                        if norm_thunks and 2 <= idx <= 9:
                            norm_thunks.pop(0)()
                    for ent in inflight:
                        emit_pv(ent)
                    queue_norm({"pr": pr, "nch": nch, "pva": pva, "pvb": pvb})
            while norm_thunks:
                norm_thunks.pop(0)()

            # ---- output projection (ot pairs: 4 open accumulators) ----
            for op2 in range(4):
                pss = [
                    psp.tile([128, 512], F32, tag=("acca", "accb")[nch], bufs=2, name="psp")
                    for j in range(2)
                    for nch in range(2)
                ]
                for dd in range(8):
                    for j in range(2):
                        ot = op2 * 2 + j
                        for nch in range(2):
                            nc.tensor.matmul(
                                pss[j * 2 + nch][:, :],
                                wp_sb[:, dd, ot * 128 : (ot + 1) * 128],
                                A_sb[dd][:, nch * 512 : (nch + 1) * 512],
                                start=(dd == 0),
                                stop=(dd == 7),
                            )
                for j in range(2):
                    ot = op2 * 2 + j
                    for nch in range(2):
                        y = wk2.tile([128, 512], F32, tag="y", bufs=3, name="y")
                        nc.vector.tensor_scalar_add(
                            y[:, :], pss[j * 2 + nch][:, :],
                            bp_sb[:, ot : ot + 1],
                        )
                        nc.scalar.dma_start(
                            yT[ot * 128 : (ot + 1) * 128, nch * 512 : (nch + 1) * 512],
                            y[:, :],
                        )
            wk2.release()

    nc.compile()
    return nc


def kernel(x, w_qkv, b_qkv, w_proj, b_proj):
    global LAST_RESULTS
    bf = ml_dtypes.bfloat16
    x = np.asarray(x, np.float32)
    w_qkv = np.asarray(w_qkv, np.float32)
    b_qkv = np.asarray(b_qkv, np.float32)
    w_proj = np.asarray(w_proj, np.float32)
    b_proj = np.asarray(b_proj, np.float32)

    wqkvT = np.ascontiguousarray(w_qkv.T.astype(bf))            # [1024, 3072]
    wprojT = np.ascontiguousarray(w_proj.T.astype(bf))          # [1024, 1024]
    bqk = np.ascontiguousarray(
        b_qkv[: 2 * C].reshape(16, 128).T.astype(np.float32)
    )                                                           # [128, 16]
    bv = np.ascontiguousarray(b_qkv[None, 2 * C :].astype(bf))  # [1, 1024]
    bproj = np.ascontiguousarray(
        b_proj.reshape(8, 128).T.astype(np.float32)
    )                                                           # [128, 8]

    in_maps = []
    for core in range(NCORES):
        b, half = core // 2, core % 2
        own = x[b][half * NQ : (half + 1) * NQ]                 # [1024, 1024]
        in_maps.append(
            {
                "xoT": np.ascontiguousarray(own.T.astype(bf)),
                "wqkvT": wqkvT,
                "bqk": bqk,
                "bv": bv,
                "wprojT": wprojT,
                "bproj": bproj,
            }
        )

    if "nc" not in _CACHE:
        _CACHE["nc"] = _build()
    nc = _CACHE["nc"]

    res = run_bass_kernel_spmd(nc, in_maps, core_ids=list(range(NCORES)))
    LAST_RESULTS = res

    out = np.empty((B, N, C), np.float32)
    for core in range(NCORES):
        b, half = core // 2, core % 2
        out[b, half * NQ : (half + 1) * NQ, :] = res.results[core]["yT"].T
    return out


if __name__ == "__main__":
    rng = np.random.default_rng(0)
    s = C ** -0.5
    ins = {
        "x": rng.standard_normal((B, N, C)).astype(np.float32),
        "w_qkv": (rng.standard_normal((3 * C, C)) * s).astype(np.float32),
        "b_qkv": (rng.standard_normal(3 * C) * 0.02).astype(np.float32),
        "w_proj": (rng.standard_normal((C, C)) * s).astype(np.float32),
        "b_proj": (rng.standard_normal(C) * 0.02).astype(np.float32),
    }
    y = kernel(**ins)
    print("out", y.shape, y.dtype, float(np.abs(y).mean()))

